# revision 30
# baseline (speedup 1.0000x reference)
"""ChannelCrossAttention TRN2 Bass kernel.

Reference computation (per batch b):
    q = Wq @ f1 + bq          [C8, N]
    k = Wk @ f2 + bk          [C8, N]
    v = Wv @ f2 + bv          [C, N]
    energy[m, n] = q[:, m] . k[:, n]
    attn = softmax over keys n
    out[c, m] = sum_n v[c, n] attn[m, n]
    result = gamma * out + f1

Sharding: 8 cores; core i handles batch b = i // 2, query half h = i % 2
(2048 query positions each). Full feat2[b] (keys/values) per core.

Kernel structure (per core):
  - f2 is loaded and tf32-rounded in 1024-column pieces so K/V projections
    start while later pieces are still in flight.
  - Q/K are built 4x-replicated across partition blocks (Q4/K4 [128, m]),
    enabling row-packed energy matmuls (K=32 contraction per row group).
  - energyT in [n(partition), m(free)] layout, f32r; exp on ScalarE over
    [128, 1024] PSUM pairs; no max subtraction (|energy| <= ~45 << 88).
  - exp output, V^T and the softmax-denominator ones vectors are bf16:
    16-bit stationaries use the fast weight-load path, which the 4-byte
    f32r stationaries cannot (fp32_mode=HIGH disables FWL), halving the
    weight-load cost of the dominant out-matmul stream.
  - softmax denominator S[m] via ones-vector matmuls (ping-ponged
    stationaries: identical consecutive stationaries serialize the PE).
  - out[c, m] += VT[n, c].T @ expT accumulated over n chunks; the
    gamma/S normalization matmul of each m-tile is deferred into the next
    m-tile's stream so the PE does not stall on the reciprocal chain.

Heavy f32 matmuls run in float32r (tf32-like, 1 col/cycle vs 4 for fp32);
float32r operands must be produced by a compute engine, so DMA'd inputs get
one DVE rounding pass.
"""

import numpy as np

B, C, H, W = 4, 256, 64, 64
N = H * W            # 4096 keys
C8 = C // 8          # 32
P = 128              # partitions
M = N // 2           # 2048 queries per core
MT = 512             # query tile (PSUM bank = 512 fp32)
NMT = M // MT        # 4
NJ = N // P          # 32 key chunks
CCH = C // P         # 2 channel chunks
NCORES = 8
FP = 1024            # f2 pipeline piece (columns)
NPC = N // FP        # 4 pieces per channel chunk

_cache = {}


def _build_nc():
    import concourse.tile as tile
    from concourse import bacc, mybir

    f32 = mybir.dt.float32
    f32r = mybir.dt.float32r
    bf16 = mybir.dt.bfloat16
    Exp = mybir.ActivationFunctionType.Exp

    nc = bacc.Bacc("TRN2", target_bir_lowering=False, debug=False)

    d_f2 = nc.dram_tensor("f2", [C, N], f32, kind="ExternalInput").ap()
    d_f1 = nc.dram_tensor("f1s", [C, M], f32, kind="ExternalInput").ap()
    # packed weights: wq4 ci0|ci1 (256), wk4 ci0|ci1 (256), wv ci0|ci1 (512),
    # bq4 (1), bk4 (1), bvb (256), grow-in-row0 (128) => 1410 cols
    WPACK = 1410
    d_wp = nc.dram_tensor("wpack", [P, WPACK], f32, kind="ExternalInput").ap()
    d_out = nc.dram_tensor("out", [C, M], f32, kind="ExternalOutput").ap()

    with tile.TileContext(nc) as tc:
        with tc.tile_pool(name="consts", bufs=1) as consts:
            # ---- persistent SBUF tensors ----
            f2r = consts.tile([P, CCH, N], f32r)       # rounded feat2
            f1raw = consts.tile([P, CCH, M], f32)      # full-precision residual
            f1r = consts.tile([P, CCH, M], f32r)       # rounded for Q proj
            wq4_sb = consts.tile([P, CCH, P], f32r)
            wk4_sb = consts.tile([P, CCH, P], f32r)
            wv_sb = consts.tile([P, CCH, C], f32r)
            bq4_sb = consts.tile([P, 1], f32)
            bk4_sb = consts.tile([P, 1], f32)
            bvb_sb = consts.tile([P, C], f32)
            grow_sb = consts.tile([1, P], f32)
            grow2_sb = consts.tile([1, P], f32)
            ones_f32 = consts.tile([P, 1], f32)
            ones_a = consts.tile([P, 1], bf16)
            ones_b = consts.tile([P, 1], bf16)
            Q4_sb = consts.tile([P, M], f32r)
            K4_sb = consts.tile([P, N], f32r)
            VT_sb = consts.tile([P, NJ, C], bf16)

            nc.vector.memset(ones_f32, 1.0)
            nc.vector.tensor_copy(ones_a, ones_f32)
            nc.vector.tensor_copy(ones_b, ones_f32)

            with tc.tile_pool(name="stage", bufs=2) as stage, \
                 tc.tile_pool(name="proj_ps", space="PSUM", bufs=2) as pps:

                # ---- all weights in one DMA, then f2 pieces, then f1 ----
                wp = stage.tile([P, WPACK], f32, tag="wp", bufs=1, name="wp")
                nc.sync.dma_start(out=wp, in_=d_wp)
                # wk first: K projection is the first matmul consumer
                for ci in range(CCH):
                    nc.vector.tensor_copy(wk4_sb[:, ci, :],
                                          wp[:, 256 + 128 * ci:256 + 128 * (ci + 1)])

                # f2 pieces: DMA + round, interleaved with K/V projections
                for pc in range(NPC):
                    cs = slice(pc * FP, (pc + 1) * FP)
                    for ci in range(CCH):
                        st = stage.tile([P, FP], f32, tag="st", bufs=3,
                                        name="st")
                        nc.sync.dma_start(out=st, in_=d_f2[ci * P:(ci + 1) * P,
                                                           cs])
                        nc.vector.tensor_copy(f2r[:, ci, cs], st)
                    if pc == 0:
                        # remaining weight unpacks, off the K-proj critical path
                        nc.vector.tensor_copy(bk4_sb, wp[:, 1025:1026])
                        for ci in range(CCH):
                            nc.vector.tensor_copy(
                                wv_sb[:, ci, :],
                                wp[:, 512 + 256 * ci:512 + 256 * (ci + 1)])
                            nc.vector.tensor_copy(
                                wq4_sb[:, ci, :],
                                wp[:, 128 * ci:128 * (ci + 1)])
                        nc.vector.tensor_copy(bq4_sb, wp[:, 1024:1025])
                        nc.vector.tensor_copy(bvb_sb, wp[:, 1026:1282])
                        nc.vector.tensor_copy(grow_sb, wp[0:1, 1282:1410])
                        nc.vector.tensor_copy(grow2_sb, wp[0:1, 1282:1410])
                    # K4 for this piece (2 tiles of 512)
                    for h in range(FP // MT):
                        nt = slice(pc * FP + h * MT, pc * FP + (h + 1) * MT)
                        k_ps = pps.tile([P, MT], f32, tag="qk", bufs=2,
                                        name="k_ps")
                        for ci in range(CCH):
                            nc.tensor.matmul(k_ps, lhsT=wk4_sb[:, ci, :],
                                             rhs=f2r[:, ci, nt],
                                             start=(ci == 0),
                                             stop=(ci == CCH - 1))
                        nc.vector.tensor_scalar_add(K4_sb[:, nt], k_ps, bk4_sb)
                    # VT for this piece (8 chunks of 128)
                    for nj in range(pc * FP // P, (pc + 1) * FP // P):
                        v_ps = pps.tile([P, C], f32, tag="v", bufs=2,
                                        name="v_ps")
                        for ci in range(CCH):
                            nc.tensor.matmul(v_ps,
                                             lhsT=f2r[:, ci,
                                                      nj * P:(nj + 1) * P],
                                             rhs=wv_sb[:, ci, :],
                                             start=(ci == 0),
                                             stop=(ci == CCH - 1))
                        nc.vector.tensor_add(VT_sb[:, nj, :], v_ps, bvb_sb)

                # f1: load, round, Q projection
                for ci in range(CCH):
                    nc.sync.dma_start(out=f1raw[:, ci, :],
                                      in_=d_f1[ci * P:(ci + 1) * P, :])
                    nc.vector.tensor_copy(f1r[:, ci, :], f1raw[:, ci, :])
                for mt in range(NMT):
                    mss = slice(mt * MT, (mt + 1) * MT)
                    q_ps = pps.tile([P, MT], f32, tag="qk", bufs=2,
                                    name="q_ps")
                    for ci in range(CCH):
                        nc.tensor.matmul(q_ps, lhsT=wq4_sb[:, ci, :],
                                         rhs=f1r[:, ci, mss],
                                         start=(ci == 0), stop=(ci == CCH - 1))
                    nc.vector.tensor_scalar_add(Q4_sb[:, mss], q_ps, bq4_sb)

            # ---- attention main loop ----
            # PSUM banks: e (2 bufs x 2 banks = 4) + out0/out1 (2) + s (1)
            # + rg (1) = 8.
            NG = NJ // 2
            with tc.tile_pool(name="main_ps", space="PSUM", bufs=1) as mps, \
                 tc.tile_pool(name="expool", bufs=4) as expool, \
                 tc.tile_pool(name="opool", bufs=2) as opool:

                def emit_energy(g, ms):
                    e = mps.tile([P, 2, MT], f32, tag="e", bufs=2, name="e")
                    for i in range(2):
                        nj = 2 * g + i
                        nc.tensor.matmul(
                            e[:, i, :],
                            lhsT=K4_sb[32 * i:32 * (i + 1),
                                       nj * P:(nj + 1) * P],
                            rhs=Q4_sb[32 * i:32 * (i + 1), ms],
                            start=True, stop=True,
                            tile_position=(32 * i, 0),
                        )
                    return e

                deferred_tail = [None]

                for mt in range(NMT):
                    ms = slice(mt * MT, (mt + 1) * MT)
                    out_ps = []
                    for cch in range(CCH):
                        o_ps = mps.tile([P, MT], f32, tag=f"out{cch}", bufs=1,
                                        name=f"o_ps{cch}")
                        out_ps.append(o_ps)
                    s_ps = mps.tile([1, MT], f32, tag="s", bufs=1)

                    e_cur = emit_energy(0, ms)
                    exs_prev = [None]
                    for g in range(NG):
                        ex = expool.tile([P, 2, MT], bf16, tag="ex",
                                         bufs=4, name="ex")
                        nc.scalar.activation(ex, e_cur, Exp)
                        if g + 1 < NG:
                            e_cur = emit_energy(g + 1, ms)
                        for i in range(2):
                            nj = 2 * g + i
                            for cch in range(CCH):
                                nc.tensor.matmul(
                                    out_ps[cch],
                                    lhsT=VT_sb[:, nj, cch * P:(cch + 1) * P],
                                    rhs=ex[:, i, :],
                                    start=(nj == 0), stop=(nj == NJ - 1),
                                )
                        exs = expool.tile([P, MT], bf16, tag="exs",
                                          bufs=3, name="exs")
                        nc.vector.tensor_add(exs, ex[:, 0, :], ex[:, 1, :])
                        if g % 2 == 0:
                            exs_prev[0] = exs
                        else:
                            exq = expool.tile([P, MT], bf16, tag="exq",
                                              bufs=3, name="exq")
                            nc.vector.tensor_add(exq, exs_prev[0], exs)
                            nc.tensor.matmul(
                                s_ps,
                                lhsT=(ones_a if g % 4 == 1 else ones_b),
                                rhs=exq,
                                start=(g == 1), stop=(g == NG - 1),
                            )
                        if g == 5 and deferred_tail[0] is not None:
                            deferred_tail[0]()
                            deferred_tail[0] = None

                    # tail part 1 (immediate): free psum banks + reciprocal
                    u_sb = []
                    for cch in range(CCH):
                        u = opool.tile([P, MT], f32, tag=f"u{cch}", bufs=2,
                                       name=f"u{cch}")
                        nc.vector.tensor_copy(u, out_ps[cch])
                        u_sb.append(u)
                    s_sb = opool.tile([1, MT], f32, tag="s_sb", bufs=2)
                    nc.vector.tensor_copy(s_sb, s_ps)
                    srow = opool.tile([1, MT], f32, tag="srow", bufs=2)
                    scr = opool.tile([1, MT], f32, tag="scr", bufs=2)
                    nc.vector.reciprocal_approx_accurate(out=srow, in_=s_sb,
                                                         scratch=scr)

                    def make_tail(mt=mt, ms=ms, u_sb=u_sb, srow=srow):
                        def tail():
                            rg_ps = mps.tile([P, MT], f32, tag="rg", bufs=1,
                                             name="rg_ps")
                            nc.tensor.matmul(
                                rg_ps,
                                lhsT=(grow_sb if mt % 2 == 0 else grow2_sb),
                                rhs=srow, start=True, stop=True)
                            rg_sb = opool.tile([P, MT], f32, tag="rg_sb",
                                               bufs=2, name="rg_sb")
                            nc.vector.tensor_copy(rg_sb, rg_ps)
                            for cch in range(CCH):
                                t_sb = opool.tile([P, MT], f32, tag=f"t{cch}",
                                                  bufs=2, name=f"t{cch}")
                                nc.vector.tensor_mul(t_sb, u_sb[cch], rg_sb)
                                o_sb = opool.tile([P, MT], f32, tag=f"o{cch}",
                                                  bufs=2, name=f"o{cch}")
                                nc.vector.tensor_add(o_sb, t_sb,
                                                     f1raw[:, cch, ms])
                                nc.sync.dma_start(
                                    out=d_out[cch * P:(cch + 1) * P, ms],
                                    in_=o_sb)
                        return tail

                    deferred_tail[0] = make_tail()

                deferred_tail[0]()

    nc.compile()
    return nc


def _get_nc():
    if "nc" not in _cache:
        _cache["nc"] = _build_nc()
    return _cache["nc"]


def kernel(feat1, feat2, Wq, bq, Wk, bk, Wv, bv, gamma, _trace=False):
    from concourse.bass_utils import run_bass_kernel_spmd

    feat1 = np.ascontiguousarray(np.asarray(feat1, dtype=np.float32))
    feat2 = np.ascontiguousarray(np.asarray(feat2, dtype=np.float32))
    f1v = feat1.reshape(B, C, N)
    f2v = feat2.reshape(B, C, N)
    wqT = np.asarray(Wq, np.float32).T                            # [C, C8]
    wkT = np.asarray(Wk, np.float32).T
    wq4 = np.concatenate([wqT] * 4, axis=1)                       # [C, 128]
    wk4 = np.concatenate([wkT] * 4, axis=1)
    bq4 = np.tile(np.asarray(bq, np.float32), 4)[:, None]         # [128, 1]
    bk4 = np.tile(np.asarray(bk, np.float32), 4)[:, None]
    wvT = np.asarray(Wv, np.float32).T                            # [C, C]
    bvb = np.broadcast_to(np.asarray(bv, np.float32)[None, :], (P, C))
    g = float(np.asarray(gamma, np.float32).reshape(-1)[0])

    # packed weight tensor, layout must match _build_nc
    wpack = np.empty((P, 1410), dtype=np.float32)
    wpack[:, 0:128] = wq4[0:P]
    wpack[:, 128:256] = wq4[P:C]
    wpack[:, 256:384] = wk4[0:P]
    wpack[:, 384:512] = wk4[P:C]
    wpack[:, 512:768] = wvT[0:P]
    wpack[:, 768:1024] = wvT[P:C]
    wpack[:, 1024:1025] = bq4
    wpack[:, 1025:1026] = bk4
    wpack[:, 1026:1282] = bvb
    wpack[:, 1282:1410] = g

    nc = _get_nc()
    in_maps = []
    for core in range(NCORES):
        b, half = core // 2, core % 2
        m0 = half * M
        in_maps.append({
            "f2": np.ascontiguousarray(f2v[b]),
            "f1s": np.ascontiguousarray(f1v[b][:, m0:m0 + M]),
            "wpack": wpack,
        })

    res = None
    last_exc = None
    for attempt in range(3):
        try:
            res = run_bass_kernel_spmd(nc, in_maps,
                                       core_ids=list(range(NCORES)),
                                       trace=_trace)
            break
        except Exception as exc:  # transient NRT device errors: retry
            last_exc = exc
    if res is None:
        raise last_exc
    _cache["last_result"] = res

    out = np.empty((B, C, N), dtype=np.float32)
    for core in range(NCORES):
        b, half = core // 2, core % 2
        m0 = half * M
        out[b][:, m0:m0 + M] = res.results[core]["out"]
    return out.reshape(B, C, H, W)
